# revision 10
# baseline (speedup 1.0000x reference)
"""Trainium2 Bass kernel for nn_BetaBinsMassAdaptive.

Computes, for each batch element:
  logits = uid_w[uid] + iid_w[iid]            (gather from two [1M,5] tables)
  bins   = softmax(logits); edges = cumsum(bins)
  cdf    = betainc(alpha, beta, edges[:4]); mass = diff([0, cdf, 1])
Returns (mass, edges), both [B,5] float32.

betainc strategy (validated to ~8e-6 max abs err vs float64):
  mass_0 = I_tau0(a,b)        + GL4 integral over [tau0, x0]
  mass_k = GL4 integral over [x_{k-1}, x_k]          (k=1,2,3)
  mass_4 = I_{1-tau4}(b,a)    + GL4 integral over [x3, tau4]
  I_tau(p,q) at FIXED tau via the 2F1 series (N=8 terms), 1/B via Stirling.
All integrands evaluated as exp((a-1)ln t + (b-1)ln(1-t)) with the 1/B(a,b)
factor folded into the final scale.

Sharding: batch split evenly across 8 NeuronCores; embedding tables are
replicated (each core gathers its own rows from HBM via indirect DMA).
"""
import sys

sys.path.insert(0, "/opt/trn_rl_repo")

import numpy as np

import concourse.bass as bass
import concourse.bacc as bacc
import concourse.mybir as mybir
from concourse.tile import TileContext
from concourse import bass_utils

F32 = mybir.dt.float32
I32 = mybir.dt.int32
AF = mybir.ActivationFunctionType
OP = mybir.AluOpType

P = 128
N_CORES = 8
B_TOTAL = 4_194_304
PER_CORE = B_TOTAL // N_CORES      # 524288
NROWS = 1_000_000
D = 5

# tunables
W = 512                             # elements per partition per chunk
CHUNK = P * W                       # 65536
NCHUNK = PER_CORE // CHUNK          # 8
TAU0 = 0.107
TAU4 = 1.0 - TAU0
NSER = 8                            # 2F1 series terms
QT = 4                              # tail GL points
QI = 4                              # interior GL points

GL_X = {
    3: [-0.7745966692414834, 0.0, 0.7745966692414834],
    4: [-0.8611363115940526, -0.33998104358485626, 0.33998104358485626, 0.8611363115940526],
    5: [-0.9061798459386640, -0.5384693101056831, 0.0, 0.5384693101056831, 0.9061798459386640],
}
GL_W = {
    3: [0.5555555555555556, 0.8888888888888888, 0.5555555555555556],
    4: [0.34785484513745385, 0.6521451548625461, 0.6521451548625461, 0.34785484513745385],
    5: [0.23692688505618908, 0.47862867049936647, 0.5688888888888889, 0.47862867049936647, 0.23692688505618908],
}

HALF_LN_2PI = 0.9189385332046727


def _emit_chunk(nc, pool, dram, c):
    """Emit instructions for one chunk of CHUNK elements."""
    v = nc.vector
    s = nc.scalar

    uid_d, iid_d, al_d, be_d, uw_d, iw_d, mass_d, edges_d = dram
    sl = slice(c * CHUNK, (c + 1) * CHUNK)

    io_tags = {"idxU", "idxI", "al", "be", "U", "V", "mass", "edges"}

    def t(name, width=W, dtype=F32):
        return pool.tile([P, width], dtype, name=name, tag=name,
                         bufs=2 if name in io_tags else 1)

    # ---- loads ----
    idxU = t("idxU", W, I32)
    idxI = t("idxI", W, I32)
    al = t("al")
    be = t("be")
    nc.sync.dma_start(idxU[:], uid_d[sl].rearrange("(p w) -> p w", p=P))
    nc.sync.dma_start(idxI[:], iid_d[sl].rearrange("(p w) -> p w", p=P))
    nc.sync.dma_start(al[:], al_d[sl].rearrange("(p w) -> p w", p=P))
    nc.sync.dma_start(be[:], be_d[sl].rearrange("(p w) -> p w", p=P))

    # HW indirect DMA consumes exactly one index per destination partition-row
    # (wide index APs are silently mis-consumed), so gather 128 rows per call.
    U = t("U", W * D)
    V = t("V", W * D)
    for w in range(W):
        nc.gpsimd.indirect_dma_start(
            out=U[:, w * D:(w + 1) * D], out_offset=None, in_=uw_d[:],
            in_offset=bass.IndirectOffsetOnAxis(ap=idxU[:, w:w + 1], axis=0))
    for w in range(W):
        nc.gpsimd.indirect_dma_start(
            out=V[:, w * D:(w + 1) * D], out_offset=None, in_=iw_d[:],
            in_offset=bass.IndirectOffsetOnAxis(ap=idxI[:, w:w + 1], axis=0))

    # ---- softmax -> edges ----
    E = t("E", W * D)
    v.tensor_tensor(out=E[:], in0=U[:], in1=V[:], op=OP.add)
    s.activation(E[:], E[:], AF.Exp)
    Ev = E[:].rearrange("p (w d) -> p w d", d=D)

    cum = t("cum", W * D)
    cumv = cum[:].rearrange("p (w d) -> p w d", d=D)
    s.activation(cumv[:, :, 0], Ev[:, :, 0], AF.Copy)
    for j in range(1, D):
        v.tensor_tensor(out=cumv[:, :, j], in0=cumv[:, :, j - 1], in1=Ev[:, :, j], op=OP.add)

    invS = t("invS")
    scr = t("scr")
    v.reciprocal_approx_accurate(out=invS[:], in_=cumv[:, :, D - 1], scratch=scr[:])

    edges = t("edges", W * D)
    edv = edges[:].rearrange("p (w d) -> p w d", d=D)
    invSb = invS[:].rearrange("p (w o) -> p w o", o=1).broadcast_to([P, W, D])
    v.tensor_tensor(out=edv[:], in0=cumv[:], in1=invSb, op=OP.mult)

    def x(k):
        return edv[:, :, k]

    # ---- per-element prep ----
    ab = t("ab")
    v.tensor_tensor(out=ab[:], in0=al[:], in1=be[:], op=OP.add)
    A1 = t("A1")
    B1 = t("B1")
    s.activation(A1[:], al[:], AF.Identity, bias=-1.0)
    s.activation(B1[:], be[:], AF.Identity, bias=-1.0)
    inv_a = t("inv_a")
    inv_b = t("inv_b")
    v.reciprocal_approx_fast(out=inv_a[:], in_=al[:])
    v.reciprocal_approx_fast(out=inv_b[:], in_=be[:])

    # ---- lnB(a,b) via Stirling shift-2 ----
    # S(w) = (w-.5)ln w - w + C + (1/12)/w - (1/360)/w^3, w = z+2
    def stirling(z, out):
        lw = t("lw")
        wt = t("wt")
        iw = t("iw")
        t1 = t("t1")
        u = t("u2")
        s.activation(lw[:], z[:], AF.Ln, bias=2.0)
        s.activation(wt[:], z[:], AF.Identity, bias=2.0)
        v.reciprocal_approx_fast(out=iw[:], in_=wt[:])
        v.scalar_tensor_tensor(out=t1[:], in0=z[:], scalar=1.5, in1=lw[:], op0=OP.add, op1=OP.mult)
        v.tensor_tensor(out=u[:], in0=iw[:], in1=iw[:], op=OP.mult)
        v.tensor_scalar(out=u[:], in0=u[:], scalar1=-1.0 / 360.0, scalar2=1.0 / 12.0, op0=OP.mult, op1=OP.add)
        v.tensor_tensor(out=u[:], in0=iw[:], in1=u[:], op=OP.mult)
        v.tensor_tensor(out=t1[:], in0=t1[:], in1=wt[:], op=OP.subtract)
        v.scalar_tensor_tensor(out=out[:], in0=u[:], scalar=HALF_LN_2PI, in1=t1[:], op0=OP.add, op1=OP.add)

    Sa = t("Sa")
    Sb = t("Sb")
    Sab = t("Sab")
    stirling(al, Sa)
    stirling(be, Sb)
    stirling(ab, Sab)
    lnB = t("lnB")
    v.tensor_tensor(out=lnB[:], in0=Sa[:], in1=Sb[:], op=OP.add)
    v.tensor_tensor(out=lnB[:], in0=lnB[:], in1=Sab[:], op=OP.subtract)
    # corr = ln(ab(ab+1)) - ln(a(a+1)) - ln(b(b+1))
    pa = t("pa")
    pb = t("pb")
    pab = t("pab")
    v.scalar_tensor_tensor(out=pa[:], in0=al[:], scalar=1.0, in1=al[:], op0=OP.add, op1=OP.mult)
    v.scalar_tensor_tensor(out=pb[:], in0=be[:], scalar=1.0, in1=be[:], op0=OP.add, op1=OP.mult)
    v.scalar_tensor_tensor(out=pab[:], in0=ab[:], scalar=1.0, in1=ab[:], op0=OP.add, op1=OP.mult)
    s.activation(pa[:], pa[:], AF.Ln)
    s.activation(pb[:], pb[:], AF.Ln)
    s.activation(pab[:], pab[:], AF.Ln)
    v.tensor_tensor(out=lnB[:], in0=lnB[:], in1=pab[:], op=OP.add)
    v.tensor_tensor(out=pa[:], in0=pa[:], in1=pb[:], op=OP.add)
    v.tensor_tensor(out=lnB[:], in0=lnB[:], in1=pa[:], op=OP.subtract)
    iB = t("iB")
    s.activation(iB[:], lnB[:], AF.Exp, scale=-1.0)

    # ---- tail series: phi = front * 2F1(1, p+q; p+1; tau), front = tau^p (1-tau)^q / p
    def tail_series(p_pl, q_pl, invp, tau, out):
        lt = float(np.log(tau))
        l1t = float(np.log1p(-tau))
        e1 = t("e1")
        tt = t("tt")
        un = t("un")
        iu = t("iu")
        v.tensor_scalar(out=e1[:], in0=p_pl[:], scalar1=lt, scalar2=None, op0=OP.mult)
        v.scalar_tensor_tensor(out=e1[:], in0=q_pl[:], scalar=l1t, in1=e1[:], op0=OP.mult, op1=OP.add)
        s.activation(e1[:], e1[:], AF.Exp)
        v.tensor_tensor(out=tt[:], in0=e1[:], in1=invp[:], op=OP.mult)   # t0 = front
        s.activation(out[:], tt[:], AF.Copy)                              # s = front
        for n in range(NSER):
            s.activation(un[:], p_pl[:], AF.Identity, scale=1.0 / tau, bias=(1.0 + n) / tau)
            v.reciprocal_approx_fast(out=iu[:], in_=un[:])
            v.tensor_tensor(out=tt[:], in0=tt[:], in1=iu[:], op=OP.mult)
            v.scalar_tensor_tensor(out=tt[:], in0=ab[:], scalar=float(n), in1=tt[:], op0=OP.add, op1=OP.mult)
            v.tensor_tensor(out=out[:], in0=out[:], in1=tt[:], op=OP.add)

    phi0 = t("phi0")
    phi4 = t("phi4")
    tail_series(al, be, inv_a, TAU0, phi0)
    tail_series(be, al, inv_b, 1.0 - TAU4, phi4)

    # ---- GL integrals ----
    mass = t("mass", W * D)
    mav = mass[:].rearrange("p (w d) -> p w d", d=D)

    dpl = t("dpl")
    hpl = t("hpl")
    mpl = t("mpl")
    tq = t("tq")
    L1 = t("L1")
    L2 = t("L2")
    uu = t("uu")
    vv = t("vv")
    acc = t("acc")

    def gl(lo, hi, Q, k, phi):
        """integral over [lo, hi]; lo/hi AP or float const. Writes mass col k."""
        xi, wq = GL_X[Q], GL_W[Q]
        if isinstance(lo, float):
            s.activation(dpl[:], hi, AF.Identity, bias=-lo)                       # d = hi - lo
            s.activation(mpl[:], dpl[:], AF.Identity, scale=0.5, bias=lo)         # m = lo + d/2
        elif isinstance(hi, float):
            s.activation(dpl[:], lo, AF.Identity, scale=-1.0, bias=hi)            # d = hi - lo
            s.activation(mpl[:], dpl[:], AF.Identity, scale=-0.5, bias=hi)        # m = hi - d/2
        else:
            v.tensor_tensor(out=dpl[:], in0=hi, in1=lo, op=OP.subtract)
            v.scalar_tensor_tensor(out=mpl[:], in0=dpl[:], scalar=0.5, in1=lo, op0=OP.mult, op1=OP.add)
        s.activation(hpl[:], dpl[:], AF.Copy, scale=0.5)                          # h = d/2
        for q in range(Q):
            v.scalar_tensor_tensor(out=tq[:], in0=hpl[:], scalar=float(xi[q]), in1=mpl[:], op0=OP.mult, op1=OP.add)
            s.activation(L1[:], tq[:], AF.Ln)
            s.activation(L2[:], tq[:], AF.Ln, scale=-1.0, bias=1.0)
            v.tensor_tensor(out=uu[:], in0=A1[:], in1=L1[:], op=OP.mult)
            v.tensor_tensor(out=vv[:], in0=B1[:], in1=L2[:], op=OP.mult)
            v.tensor_tensor(out=uu[:], in0=uu[:], in1=vv[:], op=OP.add)
            s.activation(uu[:], uu[:], AF.Exp)
            if q == 0:
                v.tensor_scalar(out=acc[:], in0=uu[:], scalar1=float(wq[q]), scalar2=None, op0=OP.mult)
            else:
                v.scalar_tensor_tensor(out=acc[:], in0=uu[:], scalar=float(wq[q]), in1=acc[:], op0=OP.mult, op1=OP.add)
        # scale: integral = acc * h;  mass contribution *= iB
        v.tensor_tensor(out=acc[:], in0=acc[:], in1=hpl[:], op=OP.mult)
        if phi is not None:
            v.tensor_tensor(out=acc[:], in0=acc[:], in1=phi[:], op=OP.add)
        v.tensor_tensor(out=mav[:, :, k], in0=acc[:], in1=iB[:], op=OP.mult)

    gl(TAU0, x(0), QT, 0, phi0)
    gl(x(0), x(1), QI, 1, None)
    gl(x(1), x(2), QI, 2, None)
    gl(x(2), x(3), QI, 3, None)
    gl(x(3), TAU4, QT, 4, phi4)

    # ---- stores ----
    nc.sync.dma_start(mass_d[sl].rearrange("(p w) d -> p (w d)", p=P), mass[:])
    nc.sync.dma_start(edges_d[sl].rearrange("(p w) d -> p (w d)", p=P), edges[:])


def _register_consts(nc):
    vals = [-1.0, 2.0, TAU0, -TAU0, TAU4, 1.0]
    for n in range(NSER):
        vals.append((1.0 + n) / TAU0)
        vals.append((1.0 + n) / (1.0 - TAU4))
    for v0 in sorted(set(vals)):
        if (F32, v0) in nc.const_aps.aps:
            continue
        tns = nc.alloc_sbuf_tensor(f"cst_f32_{len(nc.const_aps.aps)}", [128, 1], F32)
        nc.gpsimd.memset(tns.ap(), v0)
        nc.const_aps.aps[(F32, v0)] = tns.ap()
    nc.all_engine_barrier()


def build_nc(per_core=PER_CORE, nchunk=NCHUNK, n_cores=N_CORES, nrows=NROWS):
    nc = bacc.Bacc("TRN2", target_bir_lowering=False, debug=False, num_devices=n_cores)
    _register_consts(nc)
    uid_d = nc.dram_tensor("uid", [per_core], I32, kind="ExternalInput").ap()
    iid_d = nc.dram_tensor("iid", [per_core], I32, kind="ExternalInput").ap()
    al_d = nc.dram_tensor("alpha", [per_core], F32, kind="ExternalInput").ap()
    be_d = nc.dram_tensor("beta", [per_core], F32, kind="ExternalInput").ap()
    uw_d = nc.dram_tensor("uid_w", [nrows, D], F32, kind="ExternalInput").ap()
    iw_d = nc.dram_tensor("iid_w", [nrows, D], F32, kind="ExternalInput").ap()
    mass_d = nc.dram_tensor("mass", [per_core, D], F32, kind="ExternalOutput").ap()
    edges_d = nc.dram_tensor("edges", [per_core, D], F32, kind="ExternalOutput").ap()
    dram = (uid_d, iid_d, al_d, be_d, uw_d, iw_d, mass_d, edges_d)

    io_tags = {"idxU", "idxI", "al", "be", "U", "V"}
    with TileContext(nc) as tc:
        with tc.tile_pool(name="main", bufs=2) as pool:
            for c in range(nchunk):
                _emit_chunk(nc, pool, dram, c)
    nc.compile()
    return nc


_CACHED = {}


def kernel(uid, iid, alpha, beta, uid_w, iid_w):
    uid = np.ascontiguousarray(np.asarray(uid), dtype=np.int32).reshape(-1)
    iid = np.ascontiguousarray(np.asarray(iid), dtype=np.int32).reshape(-1)
    alpha = np.ascontiguousarray(np.asarray(alpha), dtype=np.float32).reshape(-1)
    beta = np.ascontiguousarray(np.asarray(beta), dtype=np.float32).reshape(-1)
    uid_w = np.ascontiguousarray(np.asarray(uid_w), dtype=np.float32)
    iid_w = np.ascontiguousarray(np.asarray(iid_w), dtype=np.float32)
    b = uid.shape[0]
    assert b == B_TOTAL, b

    if "nc" not in _CACHED:
        _CACHED["nc"] = build_nc()
    nc = _CACHED["nc"]

    pc = PER_CORE
    in_maps = []
    for c in range(N_CORES):
        sl = slice(c * pc, (c + 1) * pc)
        in_maps.append({
            "uid": uid[sl], "iid": iid[sl],
            "alpha": alpha[sl], "beta": beta[sl],
            "uid_w": uid_w, "iid_w": iid_w,
        })
    res = bass_utils.run_bass_kernel_spmd(nc, in_maps, core_ids=list(range(N_CORES)))
    mass = np.concatenate([res.results[c]["mass"] for c in range(N_CORES)], axis=0)
    edges = np.concatenate([res.results[c]["edges"] for c in range(N_CORES)], axis=0)
    return mass, edges


def time_exec(inputs, iters=5):
    """Time repeated on-device executions with device-resident inputs.

    Returns list of per-call wall seconds (excludes host<->device transfer
    of inputs; includes axon dispatch overhead), using a non-donating jit.
    """
    import jax
    from jax.sharding import Mesh, PartitionSpec
    from jax.experimental.shard_map import shard_map
    from concourse import bass2jax

    bass2jax.install_neuronx_cc_hook()
    if "nc" not in _CACHED:
        _CACHED["nc"] = build_nc()
    nc = _CACHED["nc"]
    partition_name = nc.partition_id_tensor.name if nc.partition_id_tensor else None

    in_names, out_names, out_avals = [], [], []
    for alloc in nc.m.functions[0].allocations:
        if not isinstance(alloc, mybir.MemoryLocationSet):
            continue
        name = alloc.memorylocations[0].name
        if alloc.kind == "ExternalInput":
            if name != partition_name:
                in_names.append(name)
        elif alloc.kind == "ExternalOutput":
            out_names.append(name)
            out_avals.append(jax.core.ShapedArray(tuple(alloc.tensor_shape), mybir.dt.np(alloc.dtype)))
    n_params = len(in_names)
    all_names = in_names + out_names

    bind_names = list(all_names)
    if partition_name is not None:
        bind_names.append(partition_name)

    def _body(*args):
        operands = list(args)
        if partition_name is not None:
            operands.append(bass2jax.partition_id_tensor())
        return tuple(bass2jax._bass_exec_p.bind(
            *operands,
            out_avals=tuple(out_avals),
            in_names=tuple(bind_names),
            out_names=tuple(out_names),
            lowering_input_output_aliases=(),
            sim_require_finite=True,
            sim_require_nnan=True,
            nc=nc,
        ))

    uid = np.ascontiguousarray(np.asarray(inputs["uid"]), dtype=np.int32).reshape(N_CORES, PER_CORE)
    iid = np.ascontiguousarray(np.asarray(inputs["iid"]), dtype=np.int32).reshape(N_CORES, PER_CORE)
    alpha = np.ascontiguousarray(np.asarray(inputs["alpha"]), dtype=np.float32).reshape(N_CORES, PER_CORE)
    beta = np.ascontiguousarray(np.asarray(inputs["beta"]), dtype=np.float32).reshape(N_CORES, PER_CORE)
    uid_w = np.ascontiguousarray(np.asarray(inputs["uid_w"]), dtype=np.float32)
    iid_w = np.ascontiguousarray(np.asarray(inputs["iid_w"]), dtype=np.float32)
    per_name = {
        "uid": uid.reshape(-1), "iid": iid.reshape(-1),
        "alpha": alpha.reshape(-1), "beta": beta.reshape(-1),
        "uid_w": np.concatenate([uid_w] * N_CORES, axis=0),
        "iid_w": np.concatenate([iid_w] * N_CORES, axis=0),
        "mass": np.zeros((N_CORES * PER_CORE, D), np.float32),
        "edges": np.zeros((N_CORES * PER_CORE, D), np.float32),
    }
    devices = jax.devices()[:N_CORES]
    mesh = Mesh(np.asarray(devices), ("core",))
    specs = (PartitionSpec("core"),) * len(all_names)
    out_specs = (PartitionSpec("core"),) * len(out_names)
    fn = jax.jit(shard_map(_body, mesh=mesh, in_specs=specs, out_specs=out_specs, check_rep=False),
                 keep_unused=True)
    import time as _time
    args = [jax.device_put(per_name[n]) for n in all_names]
    outs = fn(*args)
    jax.block_until_ready(outs)
    times = []
    for _ in range(iters):
        t0 = _time.time()
        outs = fn(*args)
        jax.block_until_ready(outs)
        times.append(_time.time() - t0)
    return times


def profile_once(inputs):
    """Run once with NTFF tracing; return exec_time_ns or None."""
    uid = np.ascontiguousarray(np.asarray(inputs["uid"]), dtype=np.int32).reshape(-1)
    iid = np.ascontiguousarray(np.asarray(inputs["iid"]), dtype=np.int32).reshape(-1)
    alpha = np.ascontiguousarray(np.asarray(inputs["alpha"]), dtype=np.float32).reshape(-1)
    beta = np.ascontiguousarray(np.asarray(inputs["beta"]), dtype=np.float32).reshape(-1)
    uid_w = np.ascontiguousarray(np.asarray(inputs["uid_w"]), dtype=np.float32)
    iid_w = np.ascontiguousarray(np.asarray(inputs["iid_w"]), dtype=np.float32)
    if "nc" not in _CACHED:
        _CACHED["nc"] = build_nc()
    nc = _CACHED["nc"]
    pc = PER_CORE
    in_maps = []
    for c in range(N_CORES):
        sl = slice(c * pc, (c + 1) * pc)
        in_maps.append({
            "uid": uid[sl], "iid": iid[sl],
            "alpha": alpha[sl], "beta": beta[sl],
            "uid_w": uid_w, "iid_w": iid_w,
        })
    try:
        res = bass_utils.run_bass_kernel_spmd(
            nc, in_maps, core_ids=list(range(N_CORES)), trace=True)
        if res.profile_json is not None:
            print("profile dir:", res.profile_json)
        return res.exec_time_ns
    except Exception as e:
        print("profile attempt failed:", type(e).__name__, str(e)[:200])
        return None


# revision 17
# speedup vs baseline: 1.4354x; 1.4354x over previous
"""Trainium2 Bass kernel for nn_BetaBinsMassAdaptive.

Computes, for each batch element:
  logits = uid_w[uid] + iid_w[iid]            (gather from two [1M,5] tables)
  bins   = softmax(logits); edges = cumsum(bins)
  cdf    = betainc(alpha, beta, edges[:4]); mass = diff([0, cdf, 1])
Returns (mass, edges), both [B,5] float32.

betainc strategy (validated to ~8e-6 max abs err vs float64):
  mass_0 = I_tau0(a,b)        + GL4 integral over [tau0, x0]
  mass_k = GL4 integral over [x_{k-1}, x_k]          (k=1,2,3)
  mass_4 = I_{1-tau4}(b,a)    + GL4 integral over [x3, tau4]
  I_tau(p,q) at FIXED tau via the 2F1 series (N=8 terms), 1/B via Stirling.
All integrands evaluated as exp((a-1)ln t + (b-1)ln(1-t)) with the 1/B(a,b)
factor folded into the final scale.

Sharding: batch split evenly across 8 NeuronCores; embedding tables are
replicated (each core gathers its own rows from HBM via indirect DMA).
"""
import sys

sys.path.insert(0, "/opt/trn_rl_repo")

import numpy as np

import concourse.bass as bass
import concourse.bacc as bacc
import concourse.mybir as mybir
from concourse.tile import TileContext
from concourse import bass_utils

F32 = mybir.dt.float32
I32 = mybir.dt.int32
AF = mybir.ActivationFunctionType
OP = mybir.AluOpType

P = 128
N_CORES = 8
B_TOTAL = 4_194_304
PER_CORE = B_TOTAL // N_CORES      # 524288
NROWS = 1_000_000
D = 5

# tunables
W = 512                             # elements per partition per chunk
CHUNK = P * W                       # 65536
NCHUNK = PER_CORE // CHUNK          # 8
TAU0 = 0.107
TAU4 = 1.0 - TAU0
NSER = 8                            # 2F1 series terms
QT = 4                              # tail GL points
QI = 4                              # interior GL points

GL_X = {
    3: [-0.7745966692414834, 0.0, 0.7745966692414834],
    4: [-0.8611363115940526, -0.33998104358485626, 0.33998104358485626, 0.8611363115940526],
    5: [-0.9061798459386640, -0.5384693101056831, 0.0, 0.5384693101056831, 0.9061798459386640],
}
GL_W = {
    3: [0.5555555555555556, 0.8888888888888888, 0.5555555555555556],
    4: [0.34785484513745385, 0.6521451548625461, 0.6521451548625461, 0.34785484513745385],
    5: [0.23692688505618908, 0.47862867049936647, 0.5688888888888889, 0.47862867049936647, 0.23692688505618908],
}

HALF_LN_2PI = 0.9189385332046727


def _emit_chunk(nc, pool, dram, c):
    """Emit instructions for one chunk of CHUNK elements."""
    v = nc.vector
    s = nc.scalar

    uid_d, iid_d, al_d, be_d, uw_d, iw_d, mass_d, edges_d = dram
    sl = slice(c * CHUNK, (c + 1) * CHUNK)

    io_tags = {"idxU", "idxI", "al", "be", "U", "V", "mass", "edges"}

    def t(name, width=W, dtype=F32):
        return pool.tile([P, width], dtype, name=name, tag=name,
                         bufs=2 if name in io_tags else 1)

    # ---- loads ----
    idxU = t("idxU", W, I32)
    idxI = t("idxI", W, I32)
    al = t("al")
    be = t("be")
    nc.sync.dma_start(idxU[:], uid_d[sl].rearrange("(p w) -> p w", p=P))
    nc.sync.dma_start(idxI[:], iid_d[sl].rearrange("(p w) -> p w", p=P))
    nc.sync.dma_start(al[:], al_d[sl].rearrange("(p w) -> p w", p=P))
    nc.sync.dma_start(be[:], be_d[sl].rearrange("(p w) -> p w", p=P))

    # HW indirect DMA consumes exactly one index per destination partition-row
    # (wide index APs are silently mis-consumed), so gather 128 rows per call.
    # Stripe consecutive gathers across NSUB independent sub-tiles per table so
    # same-tile WAW ordering doesn't serialize the DMA pipeline.
    NSUB = 4
    SW = W // NSUB
    Us = [t(f"U{k}", SW * D) for k in range(NSUB)]
    Vs = [t(f"V{k}", SW * D) for k in range(NSUB)]
    for w in range(W):
        k, wl = w % NSUB, w // NSUB
        gi = nc.gpsimd.indirect_dma_start(
            out=Us[k][:, wl * D:(wl + 1) * D], out_offset=None, in_=uw_d[:],
            in_offset=bass.IndirectOffsetOnAxis(ap=idxU[:, w:w + 1], axis=0))
        gi.ins.queue = f"qPoolDynamic{k or ''}"
        gi = nc.gpsimd.indirect_dma_start(
            out=Vs[k][:, wl * D:(wl + 1) * D], out_offset=None, in_=iw_d[:],
            in_offset=bass.IndirectOffsetOnAxis(ap=idxI[:, w:w + 1], axis=0))
        gi.ins.queue = f"qPoolDynamic{k or ''}"

    # ---- softmax -> edges ----
    # element (p, w) with w = wl*NSUB + k lives in Us[k]/Vs[k] at local col wl
    E = t("E", W * D)
    Evv = E[:].rearrange("p (wl s d) -> p wl s d", s=NSUB, d=D)
    for k in range(NSUB):
        uv = Us[k][:].rearrange("p (wl d) -> p wl d", d=D)
        vv_ = Vs[k][:].rearrange("p (wl d) -> p wl d", d=D)
        v.tensor_tensor(out=Evv[:, :, k, :], in0=uv[:], in1=vv_[:], op=OP.add)
    s.activation(E[:], E[:], AF.Exp)
    Ev = E[:].rearrange("p (w d) -> p w d", d=D)

    cum = t("cum", W * D)
    cumv = cum[:].rearrange("p (w d) -> p w d", d=D)
    s.activation(cumv[:, :, 0], Ev[:, :, 0], AF.Copy)
    for j in range(1, D):
        v.tensor_tensor(out=cumv[:, :, j], in0=cumv[:, :, j - 1], in1=Ev[:, :, j], op=OP.add)

    invS = t("invS")
    scr = t("scr")
    v.reciprocal_approx_accurate(out=invS[:], in_=cumv[:, :, D - 1], scratch=scr[:])

    edges = t("edges", W * D)
    edv = edges[:].rearrange("p (w d) -> p w d", d=D)
    invSb = invS[:].rearrange("p (w o) -> p w o", o=1).broadcast_to([P, W, D])
    v.tensor_tensor(out=edv[:], in0=cumv[:], in1=invSb, op=OP.mult)

    def x(k):
        return edv[:, :, k]

    # ---- per-element prep ----
    ab = t("ab")
    v.tensor_tensor(out=ab[:], in0=al[:], in1=be[:], op=OP.add)
    A1 = t("A1")
    B1 = t("B1")
    s.activation(A1[:], al[:], AF.Identity, bias=-1.0)
    s.activation(B1[:], be[:], AF.Identity, bias=-1.0)
    inv_a = t("inv_a")
    inv_b = t("inv_b")
    v.reciprocal_approx_fast(out=inv_a[:], in_=al[:])
    v.reciprocal_approx_fast(out=inv_b[:], in_=be[:])

    # ---- lnB(a,b) via Stirling shift-2 ----
    # S(w) = (w-.5)ln w - w + C + (1/12)/w - (1/360)/w^3, w = z+2
    def stirling(z, out):
        lw = t("lw")
        wt = t("wt")
        iw = t("iw")
        t1 = t("t1")
        u = t("u2")
        s.activation(lw[:], z[:], AF.Ln, bias=2.0)
        s.activation(wt[:], z[:], AF.Identity, bias=2.0)
        v.reciprocal_approx_fast(out=iw[:], in_=wt[:])
        v.scalar_tensor_tensor(out=t1[:], in0=z[:], scalar=1.5, in1=lw[:], op0=OP.add, op1=OP.mult)
        v.tensor_tensor(out=u[:], in0=iw[:], in1=iw[:], op=OP.mult)
        v.tensor_scalar(out=u[:], in0=u[:], scalar1=-1.0 / 360.0, scalar2=1.0 / 12.0, op0=OP.mult, op1=OP.add)
        v.tensor_tensor(out=u[:], in0=iw[:], in1=u[:], op=OP.mult)
        v.tensor_tensor(out=t1[:], in0=t1[:], in1=wt[:], op=OP.subtract)
        v.scalar_tensor_tensor(out=out[:], in0=u[:], scalar=HALF_LN_2PI, in1=t1[:], op0=OP.add, op1=OP.add)

    Sa = t("Sa")
    Sb = t("Sb")
    Sab = t("Sab")
    stirling(al, Sa)
    stirling(be, Sb)
    stirling(ab, Sab)
    lnB = t("lnB")
    v.tensor_tensor(out=lnB[:], in0=Sa[:], in1=Sb[:], op=OP.add)
    v.tensor_tensor(out=lnB[:], in0=lnB[:], in1=Sab[:], op=OP.subtract)
    # corr = ln(ab(ab+1)) - ln(a(a+1)) - ln(b(b+1))
    pa = t("pa")
    pb = t("pb")
    pab = t("pab")
    v.scalar_tensor_tensor(out=pa[:], in0=al[:], scalar=1.0, in1=al[:], op0=OP.add, op1=OP.mult)
    v.scalar_tensor_tensor(out=pb[:], in0=be[:], scalar=1.0, in1=be[:], op0=OP.add, op1=OP.mult)
    v.scalar_tensor_tensor(out=pab[:], in0=ab[:], scalar=1.0, in1=ab[:], op0=OP.add, op1=OP.mult)
    s.activation(pa[:], pa[:], AF.Ln)
    s.activation(pb[:], pb[:], AF.Ln)
    s.activation(pab[:], pab[:], AF.Ln)
    v.tensor_tensor(out=lnB[:], in0=lnB[:], in1=pab[:], op=OP.add)
    v.tensor_tensor(out=pa[:], in0=pa[:], in1=pb[:], op=OP.add)
    v.tensor_tensor(out=lnB[:], in0=lnB[:], in1=pa[:], op=OP.subtract)
    iB = t("iB")
    s.activation(iB[:], lnB[:], AF.Exp, scale=-1.0)

    # ---- tail series: phi = front * 2F1(1, p+q; p+1; tau), front = tau^p (1-tau)^q / p
    def tail_series(p_pl, q_pl, invp, tau, out):
        lt = float(np.log(tau))
        l1t = float(np.log1p(-tau))
        e1 = t("e1")
        tt = t("tt")
        un = t("un")
        iu = t("iu")
        v.tensor_scalar(out=e1[:], in0=p_pl[:], scalar1=lt, scalar2=None, op0=OP.mult)
        v.scalar_tensor_tensor(out=e1[:], in0=q_pl[:], scalar=l1t, in1=e1[:], op0=OP.mult, op1=OP.add)
        s.activation(e1[:], e1[:], AF.Exp)
        v.tensor_tensor(out=tt[:], in0=e1[:], in1=invp[:], op=OP.mult)   # t0 = front
        s.activation(out[:], tt[:], AF.Copy)                              # s = front
        for n in range(NSER):
            s.activation(un[:], p_pl[:], AF.Identity, scale=1.0 / tau, bias=(1.0 + n) / tau)
            v.reciprocal_approx_fast(out=iu[:], in_=un[:])
            v.tensor_tensor(out=tt[:], in0=tt[:], in1=iu[:], op=OP.mult)
            v.scalar_tensor_tensor(out=tt[:], in0=ab[:], scalar=float(n), in1=tt[:], op0=OP.add, op1=OP.mult)
            v.tensor_tensor(out=out[:], in0=out[:], in1=tt[:], op=OP.add)

    phi0 = t("phi0")
    phi4 = t("phi4")
    tail_series(al, be, inv_a, TAU0, phi0)
    tail_series(be, al, inv_b, 1.0 - TAU4, phi4)

    # ---- GL integrals ----
    mass = t("mass", W * D)
    mav = mass[:].rearrange("p (w d) -> p w d", d=D)

    dpl = t("dpl")
    hpl = t("hpl")
    mpl = t("mpl")
    tq = t("tq")
    L1 = t("L1")
    L2 = t("L2")
    uu = t("uu")
    vv = t("vv")
    acc = t("acc")

    def gl(lo, hi, Q, k, phi):
        """integral over [lo, hi]; lo/hi AP or float const. Writes mass col k."""
        xi, wq = GL_X[Q], GL_W[Q]
        if isinstance(lo, float):
            s.activation(dpl[:], hi, AF.Identity, bias=-lo)                       # d = hi - lo
            s.activation(mpl[:], dpl[:], AF.Identity, scale=0.5, bias=lo)         # m = lo + d/2
        elif isinstance(hi, float):
            s.activation(dpl[:], lo, AF.Identity, scale=-1.0, bias=hi)            # d = hi - lo
            s.activation(mpl[:], dpl[:], AF.Identity, scale=-0.5, bias=hi)        # m = hi - d/2
        else:
            v.tensor_tensor(out=dpl[:], in0=hi, in1=lo, op=OP.subtract)
            v.scalar_tensor_tensor(out=mpl[:], in0=dpl[:], scalar=0.5, in1=lo, op0=OP.mult, op1=OP.add)
        s.activation(hpl[:], dpl[:], AF.Copy, scale=0.5)                          # h = d/2
        for q in range(Q):
            v.scalar_tensor_tensor(out=tq[:], in0=hpl[:], scalar=float(xi[q]), in1=mpl[:], op0=OP.mult, op1=OP.add)
            s.activation(L1[:], tq[:], AF.Ln)
            s.activation(L2[:], tq[:], AF.Ln, scale=-1.0, bias=1.0)
            v.tensor_tensor(out=uu[:], in0=A1[:], in1=L1[:], op=OP.mult)
            v.tensor_tensor(out=vv[:], in0=B1[:], in1=L2[:], op=OP.mult)
            v.tensor_tensor(out=uu[:], in0=uu[:], in1=vv[:], op=OP.add)
            s.activation(uu[:], uu[:], AF.Exp)
            if q == 0:
                v.tensor_scalar(out=acc[:], in0=uu[:], scalar1=float(wq[q]), scalar2=None, op0=OP.mult)
            else:
                v.scalar_tensor_tensor(out=acc[:], in0=uu[:], scalar=float(wq[q]), in1=acc[:], op0=OP.mult, op1=OP.add)
        # scale: integral = acc * h;  mass contribution *= iB
        v.tensor_tensor(out=acc[:], in0=acc[:], in1=hpl[:], op=OP.mult)
        if phi is not None:
            v.tensor_tensor(out=acc[:], in0=acc[:], in1=phi[:], op=OP.add)
        v.tensor_tensor(out=mav[:, :, k], in0=acc[:], in1=iB[:], op=OP.mult)

    gl(TAU0, x(0), QT, 0, phi0)
    gl(x(0), x(1), QI, 1, None)
    gl(x(1), x(2), QI, 2, None)
    gl(x(2), x(3), QI, 3, None)
    gl(x(3), TAU4, QT, 4, phi4)

    # ---- stores ----
    nc.sync.dma_start(mass_d[sl].rearrange("(p w) d -> p (w d)", p=P), mass[:])
    nc.sync.dma_start(edges_d[sl].rearrange("(p w) d -> p (w d)", p=P), edges[:])


def _register_consts(nc):
    vals = [-1.0, 2.0, TAU0, -TAU0, TAU4, 1.0]
    for n in range(NSER):
        vals.append((1.0 + n) / TAU0)
        vals.append((1.0 + n) / (1.0 - TAU4))
    for v0 in sorted(set(vals)):
        if (F32, v0) in nc.const_aps.aps:
            continue
        tns = nc.alloc_sbuf_tensor(f"cst_f32_{len(nc.const_aps.aps)}", [128, 1], F32)
        nc.gpsimd.memset(tns.ap(), v0)
        nc.const_aps.aps[(F32, v0)] = tns.ap()
    nc.all_engine_barrier()


def build_nc(per_core=PER_CORE, nchunk=NCHUNK, n_cores=N_CORES, nrows=NROWS):
    nc = bacc.Bacc("TRN2", target_bir_lowering=False, debug=False, num_devices=n_cores,
                   num_swdge_queues=4)
    _register_consts(nc)
    uid_d = nc.dram_tensor("uid", [per_core], I32, kind="ExternalInput").ap()
    iid_d = nc.dram_tensor("iid", [per_core], I32, kind="ExternalInput").ap()
    al_d = nc.dram_tensor("alpha", [per_core], F32, kind="ExternalInput").ap()
    be_d = nc.dram_tensor("beta", [per_core], F32, kind="ExternalInput").ap()
    uw_d = nc.dram_tensor("uid_w", [nrows, D], F32, kind="ExternalInput").ap()
    iw_d = nc.dram_tensor("iid_w", [nrows, D], F32, kind="ExternalInput").ap()
    mass_d = nc.dram_tensor("mass", [per_core, D], F32, kind="ExternalOutput").ap()
    edges_d = nc.dram_tensor("edges", [per_core, D], F32, kind="ExternalOutput").ap()
    dram = (uid_d, iid_d, al_d, be_d, uw_d, iw_d, mass_d, edges_d)

    io_tags = {"idxU", "idxI", "al", "be", "U", "V"}
    with TileContext(nc) as tc:
        with tc.tile_pool(name="main", bufs=2) as pool:
            for c in range(nchunk):
                _emit_chunk(nc, pool, dram, c)
    nc.compile()
    return nc


_CACHED = {}


def _prep_idx(arr):
    """Per-chunk transpose so gather w's 128 indices are contiguous in one
    partition: IT[r, c*128 + p] = arr[p*W + c*128 + r] (w = c*128 + r)."""
    a = arr.reshape(-1, P, W // P, P)          # (chunk, p, c, r)
    return np.ascontiguousarray(a.transpose(0, 3, 2, 1)).reshape(-1)


def kernel(uid, iid, alpha, beta, uid_w, iid_w):
    uid = np.ascontiguousarray(np.asarray(uid), dtype=np.int32).reshape(-1)
    iid = np.ascontiguousarray(np.asarray(iid), dtype=np.int32).reshape(-1)
    alpha = np.ascontiguousarray(np.asarray(alpha), dtype=np.float32).reshape(-1)
    beta = np.ascontiguousarray(np.asarray(beta), dtype=np.float32).reshape(-1)
    uid_w = np.ascontiguousarray(np.asarray(uid_w), dtype=np.float32)
    iid_w = np.ascontiguousarray(np.asarray(iid_w), dtype=np.float32)
    b = uid.shape[0]
    assert b == B_TOTAL, b

    if "nc" not in _CACHED:
        _CACHED["nc"] = build_nc()
    nc = _CACHED["nc"]

    pc = PER_CORE
    in_maps = []
    for c in range(N_CORES):
        sl = slice(c * pc, (c + 1) * pc)
        in_maps.append({
            "uid": uid[sl], "iid": iid[sl],
            "alpha": alpha[sl], "beta": beta[sl],
            "uid_w": uid_w, "iid_w": iid_w,
        })
    res = bass_utils.run_bass_kernel_spmd(nc, in_maps, core_ids=list(range(N_CORES)))
    mass = np.concatenate([res.results[c]["mass"] for c in range(N_CORES)], axis=0)
    edges = np.concatenate([res.results[c]["edges"] for c in range(N_CORES)], axis=0)
    return mass, edges


def time_exec(inputs, iters=5):
    """Time repeated on-device executions with device-resident inputs.

    Returns list of per-call wall seconds (excludes host<->device transfer
    of inputs; includes axon dispatch overhead), using a non-donating jit.
    """
    import jax
    from jax.sharding import Mesh, PartitionSpec
    from jax.experimental.shard_map import shard_map
    from concourse import bass2jax

    bass2jax.install_neuronx_cc_hook()
    if "nc" not in _CACHED:
        _CACHED["nc"] = build_nc()
    nc = _CACHED["nc"]
    partition_name = nc.partition_id_tensor.name if nc.partition_id_tensor else None

    in_names, out_names, out_avals = [], [], []
    for alloc in nc.m.functions[0].allocations:
        if not isinstance(alloc, mybir.MemoryLocationSet):
            continue
        name = alloc.memorylocations[0].name
        if alloc.kind == "ExternalInput":
            if name != partition_name:
                in_names.append(name)
        elif alloc.kind == "ExternalOutput":
            out_names.append(name)
            out_avals.append(jax.core.ShapedArray(tuple(alloc.tensor_shape), mybir.dt.np(alloc.dtype)))
    n_params = len(in_names)
    all_names = in_names + out_names

    bind_names = list(all_names)
    if partition_name is not None:
        bind_names.append(partition_name)

    def _body(*args):
        operands = list(args)
        if partition_name is not None:
            operands.append(bass2jax.partition_id_tensor())
        return tuple(bass2jax._bass_exec_p.bind(
            *operands,
            out_avals=tuple(out_avals),
            in_names=tuple(bind_names),
            out_names=tuple(out_names),
            lowering_input_output_aliases=(),
            sim_require_finite=True,
            sim_require_nnan=True,
            nc=nc,
        ))

    uid = np.ascontiguousarray(np.asarray(inputs["uid"]), dtype=np.int32).reshape(N_CORES, PER_CORE)
    iid = np.ascontiguousarray(np.asarray(inputs["iid"]), dtype=np.int32).reshape(N_CORES, PER_CORE)
    alpha = np.ascontiguousarray(np.asarray(inputs["alpha"]), dtype=np.float32).reshape(N_CORES, PER_CORE)
    beta = np.ascontiguousarray(np.asarray(inputs["beta"]), dtype=np.float32).reshape(N_CORES, PER_CORE)
    uid_w = np.ascontiguousarray(np.asarray(inputs["uid_w"]), dtype=np.float32)
    iid_w = np.ascontiguousarray(np.asarray(inputs["iid_w"]), dtype=np.float32)
    per_name = {
        "uid": uid.reshape(-1), "iid": iid.reshape(-1),
        "alpha": alpha.reshape(-1), "beta": beta.reshape(-1),
        "uid_w": np.concatenate([uid_w] * N_CORES, axis=0),
        "iid_w": np.concatenate([iid_w] * N_CORES, axis=0),
        "mass": np.zeros((N_CORES * PER_CORE, D), np.float32),
        "edges": np.zeros((N_CORES * PER_CORE, D), np.float32),
    }
    devices = jax.devices()[:N_CORES]
    mesh = Mesh(np.asarray(devices), ("core",))
    specs = (PartitionSpec("core"),) * len(all_names)
    out_specs = (PartitionSpec("core"),) * len(out_names)
    fn = jax.jit(shard_map(_body, mesh=mesh, in_specs=specs, out_specs=out_specs, check_rep=False),
                 keep_unused=True)
    import time as _time
    args = [jax.device_put(per_name[n]) for n in all_names]
    outs = fn(*args)
    jax.block_until_ready(outs)
    times = []
    for _ in range(iters):
        t0 = _time.time()
        outs = fn(*args)
        jax.block_until_ready(outs)
        times.append(_time.time() - t0)
    return times


def profile_once(inputs):
    """Run once with NTFF tracing; return exec_time_ns or None."""
    uid = np.ascontiguousarray(np.asarray(inputs["uid"]), dtype=np.int32).reshape(-1)
    iid = np.ascontiguousarray(np.asarray(inputs["iid"]), dtype=np.int32).reshape(-1)
    alpha = np.ascontiguousarray(np.asarray(inputs["alpha"]), dtype=np.float32).reshape(-1)
    beta = np.ascontiguousarray(np.asarray(inputs["beta"]), dtype=np.float32).reshape(-1)
    uid_w = np.ascontiguousarray(np.asarray(inputs["uid_w"]), dtype=np.float32)
    iid_w = np.ascontiguousarray(np.asarray(inputs["iid_w"]), dtype=np.float32)
    if "nc" not in _CACHED:
        _CACHED["nc"] = build_nc()
    nc = _CACHED["nc"]
    pc = PER_CORE
    in_maps = []
    for c in range(N_CORES):
        sl = slice(c * pc, (c + 1) * pc)
        in_maps.append({
            "uid": uid[sl], "iid": iid[sl],
            "alpha": alpha[sl], "beta": beta[sl],
            "uid_w": uid_w, "iid_w": iid_w,
        })
    try:
        res = bass_utils.run_bass_kernel_spmd(
            nc, in_maps, core_ids=list(range(N_CORES)), trace=True)
        if res.profile_json is not None:
            print("profile dir:", res.profile_json)
        return res.exec_time_ns
    except Exception as e:
        print("profile attempt failed:", type(e).__name__, str(e)[:200])
        return None
